# revision 1
# baseline (speedup 1.0000x reference)
"""Trainium2 Bass kernel for DeBERTa-style disentangled attention.

Problem: B=16, S=512, D=768, H=12, HD=64, L=512 (att_span), scale=sqrt(3*64).

  Q = q@Wq+bq, K = k@Wk+bk, V = v@Wv+bv   (per-head split)
  scores = (QK^T + c2p + p2c) / scale ; softmax ; ctx = P@V
  c2p[i,j] = Q[i] . pos_k[i-j+512]   (pos_k = rel@Wk+bk, per head)
  p2c[i,j] = K[j] . pos_q[i-j+512]   (pos_q = rel@Wq+bq)
  (clip never binds: i-j+512 in [1,1023])

Sharding: data-parallel over batch, 8 cores x (B_local=2).

Device strategy (per core, everything transposed "scores^T[j,i]"):
  - Projections produce QT/KT [dout, tok] (bf16), V [tok, dout] (bf16,
    augmented with a ones-column per head for softmax denominators),
    PKR = pos_k_reversed^T and PQ = pos_q^T [dout, p] (bf16).
    1/scale folded into Wq/bq on host (covers c2c, c2p via Q; p2c via pos_q).
  - Per (b,h): band matmuls produce c2p_att_rev / p2c_att [128, 640] tiles,
    evacuated bf16 and round-tripped through DRAM; strided re-read
    (row-stride 639, offset 127) yields the diagonal-gathered
    c2p [i,j] / p2cT [j,i] tiles (the DeBERTa "skew" trick).
  - scores^T accumulated in PSUM: c2cT matmul + p2cT via identity-add +
    c2p via PE add-transpose (lhsT=c2p chunk, rhs=identity).
  - exp on ACT (no max subtraction needed: |scores| <~ 3), PV matmul with
    ones-augmented V gives ctx^T and row sums; PE transpose + DVE
    reciprocal/scale finishes ctx = P@V / sums in fp32.
"""

import os
import sys
import numpy as np

for p in ("/opt/trn_rl_repo",):
    if p not in sys.path:
        sys.path.insert(0, p)

import ml_dtypes

import concourse.bass as bass
import concourse.bacc as bacc
import concourse.tile as tile
import concourse.mybir as mybir
from concourse import bass_utils

f32 = mybir.dt.float32
bf16 = mybir.dt.bfloat16
fp8 = mybir.dt.float8e4
FT = mybir.ActivationFunctionType

B, S, D, H = 16, 512, 768, 12
HD = 64
L = 512
P2 = 2 * L  # 1024
NB = 2  # batches per core
NTOK = NB * S  # 1024
NCORES = 8
SCALE = float(np.sqrt(HD * 3.0))
KC = D // 128  # 6 contraction chunks
BW = 640  # band width (pads the 639 used diagonals)
BP = 639  # band row pitch for the skew read

_nbf = ml_dtypes.bfloat16

# ablation / tuning knobs (TimelineSim experiments)
_ABL = set(os.environ.get("KABL", "").split(",")) - {""}
_BUFS = {}
for _kv in os.environ.get("KBUFS", "").split(","):
    if _kv:
        _k, _v = _kv.split("=")
        _BUFS[_k] = int(_v)


def _bufs(name, default):
    return _BUFS.get(name, default)


def build_kernel(abl=None, bufs=None, nrep=1):
    global _ABL, _BUFS
    if abl is not None:
        _ABL = set(abl)
    if bufs is not None:
        _BUFS = dict(bufs)
    nc = bacc.Bacc(
        "TRN2",
        target_bir_lowering=False,
        debug=False,
        enable_asserts=False,
        num_devices=NCORES,
    )

    # ---- I/O ----
    qT_d = nc.dram_tensor("qT", [D, NTOK], bf16, kind="ExternalInput")
    kT_d = nc.dram_tensor("kT", [D, NTOK], bf16, kind="ExternalInput")
    vT_d = nc.dram_tensor("vT", [D, NTOK], bf16, kind="ExternalInput")
    wq_d = nc.dram_tensor("Wq", [D, D], bf16, kind="ExternalInput")  # pre-scaled
    wk_d = nc.dram_tensor("Wk", [D, D], bf16, kind="ExternalInput")
    wv_d = nc.dram_tensor("Wv", [D, D], bf16, kind="ExternalInput")
    rT_d = nc.dram_tensor("rT", [D, P2], bf16, kind="ExternalInput")
    rTr_d = nc.dram_tensor("rTrev", [D, P2], bf16, kind="ExternalInput")
    bias_d = nc.dram_tensor("biases", [128, 2 * KC], f32, kind="ExternalInput")
    out_d = nc.dram_tensor("out", [NTOK, D], f32, kind="ExternalOutput")

    idn_np = np.eye(128, dtype=_nbf)
    if "fp8rt" in _ABL:
        idn_np = np.eye(128, dtype=ml_dtypes.float8_e4m3)
    idn_d = nc.inline_tensor(idn_np, name="idn_bf")
    idnf_np = np.eye(128, dtype=np.float32)
    idnf_d = nc.inline_tensor(idnf_np, name="idn_f32")

    with tile.TileContext(nc) as tc:
        for _rep in range(nrep):
            _body(nc, tc, qT_d, kT_d, vT_d, wq_d, wk_d, wv_d, rT_d, rTr_d,
                  bias_d, idn_d, idnf_d, out_d)
    nc.compile()
    return nc


def _body(nc, tc, qT_d, kT_d, vT_d, wq_d, wk_d, wv_d, rT_d, rTr_d,
          bias_d, idn_d, idnf_d, out_d):
    from contextlib import ExitStack

    with ExitStack() as big:
        const = big.enter_context(tc.tile_pool(name="const", bufs=1))
        acts = big.enter_context(tc.tile_pool(name="acts", bufs=1))

        bdt = fp8 if "fp8rt" in _ABL else bf16
        idn = const.tile([128, 128], bdt)
        nc.sync.dma_start(idn[:], idn_d.ap())
        idnf = const.tile([128, 128], f32)
        nc.sync.dma_start(idnf[:], idnf_d.ap())
        biases = const.tile([128, 2 * KC], f32)
        nc.sync.dma_start(biases[:], bias_d.ap())

        # persistent activations
        QT = [acts.tile([128, NTOK], bf16, name=f"QT{t}") for t in range(KC)]
        KT = [acts.tile([128, NTOK], bf16, name=f"KT{t}") for t in range(KC)]
        PKR = [acts.tile([128, P2 + 1], bf16, name=f"PKR{t}") for t in range(KC)]
        PQ = [acts.tile([128, P2 + 1], bf16, name=f"PQ{t}") for t in range(KC)]
        VA = [acts.tile([128, 65 * H], bf16, name=f"VA{c}") for c in range(8)]

        # ---------------- Stage P: projections ----------------
        with ExitStack() as st:
            inp = st.enter_context(tc.tile_pool(name="inp", bufs=1))
            psp = st.enter_context(
                tc.tile_pool(name="psp", bufs=4, space="PSUM"))

            qT = [inp.tile([128, NTOK], bf16, name=f"qT{t}") for t in range(KC)]
            kTt = [inp.tile([128, NTOK], bf16, name=f"kTt{t}") for t in range(KC)]
            vT = [inp.tile([128, NTOK], bf16, name=f"vT{t}") for t in range(KC)]
            rT = [inp.tile([128, P2], bf16, name=f"rT{t}") for t in range(KC)]
            rTr = [inp.tile([128, P2], bf16, name=f"rTr{t}") for t in range(KC)]
            for t in range(KC):
                sl = slice(128 * t, 128 * (t + 1))
                nc.sync.dma_start(qT[t][:], qT_d.ap()[sl])
                nc.sync.dma_start(kTt[t][:], kT_d.ap()[sl])
                nc.sync.dma_start(vT[t][:], vT_d.ap()[sl])
                nc.sync.dma_start(rT[t][:], rT_d.ap()[sl])
                nc.sync.dma_start(rTr[t][:], rTr_d.ap()[sl])

            wq = [inp.tile([128, D], bf16, name=f"wq{t}") for t in range(KC)]
            wk = [inp.tile([128, D], bf16, name=f"wk{t}") for t in range(KC)]
            wv = [inp.tile([128, D], bf16, name=f"wv{t}") for t in range(KC)]
            for t in range(KC):
                sl = slice(128 * t, 128 * (t + 1))
                nc.sync.dma_start(wq[t][:], wq_d.ap()[sl])
                nc.sync.dma_start(wk[t][:], wk_d.ap()[sl])
                nc.sync.dma_start(wv[t][:], wv_d.ap()[sl])

            # QT / KT / PKR / PQ : out[dout_tile, tok] = W^T @ xT (+ bias)
            if "noproj" in _ABL:
                for t in range(KC):
                    nc.vector.memset(QT[t][:], 0.0)
                    nc.vector.memset(KT[t][:], 0.0)
                    nc.vector.memset(PKR[t][:], 0.0)
                    nc.vector.memset(PQ[t][:], 0.0)
                for c in range(8):
                    nc.vector.memset(VA[c][:], 0.0)
            for t in range(KC if "noproj" not in _ABL else 0):
                wsl = slice(128 * t, 128 * (t + 1))
                for th in range(2):  # token/pos halves of 512
                    tsl = slice(512 * th, 512 * (th + 1))
                    for (wmat, xin, bcol, dst) in (
                        (wq, qT, 0, QT), (wk, kTt, 1, KT),
                        (wk, rTr, 1, PKR), (wq, rT, 0, PQ),
                    ):
                        ps = psp.tile([128, 512], f32, name="ps_proj",
                                      tag="ps_proj", bufs=4)
                        for kc in range(KC):
                            nc.tensor.matmul(
                                ps[:], wmat[kc][:, wsl], xin[kc][:, tsl],
                                start=(kc == 0), stop=(kc == KC - 1))
                        nc.scalar.activation(
                            dst[t][:, tsl], ps[:], FT.Identity,
                            bias=biases[:, bcol * KC + t : bcol * KC + t + 1],
                            scale=1.0)

            # garbage-pad column P2 of PKR/PQ: zero it
            for t in range(KC):
                nc.vector.memset(PKR[t][:, P2:P2 + 1], 0.0)
                nc.vector.memset(PQ[t][:, P2:P2 + 1], 0.0)

            # V (+ ones cols): out[tok_chunk, dout] = vT^T @ Wv
            for c in range(8 if "noproj" not in _ABL else 0):
                csl = slice(128 * c, 128 * (c + 1))
                ps = psp.tile([128, D], f32, name="ps_v", tag="ps_v", bufs=2)
                for osl in (slice(0, 512), slice(512, D)):
                    for kc in range(KC):
                        nc.tensor.matmul(
                            ps[:, osl], vT[kc][:, csl], wv[kc][:, osl],
                            start=(kc == 0), stop=(kc == KC - 1))
                # strided evac: VA[c][:, 65h + d] = ps[:, 64h + d]
                va_v = VA[c][:].rearrange("p (h c) -> p h c", c=65)
                ps_v = ps[:].rearrange("p (h c) -> p h c", c=64)
                nc.vector.tensor_copy(va_v[:, :, 0:64], ps_v)
                nc.vector.memset(va_v[:, :, 64:65], 1.0)

        # ---------------- Stage A: attention ----------------
        with ExitStack() as st:
            dram = st.enter_context(
                tc.tile_pool(name="dramb", bufs=_bufs("dramb", 2), space="DRAM"))
            bsb = st.enter_context(
                tc.tile_pool(name="bsb", bufs=_bufs("bsb", 12)))
            brd = st.enter_context(
                tc.tile_pool(name="brd", bufs=_bufs("brd", 3)))
            expp = st.enter_context(
                tc.tile_pool(name="expp", bufs=_bufs("expp", 2)))
            outp = st.enter_context(tc.tile_pool(name="outp", bufs=2))
            smal = st.enter_context(tc.tile_pool(name="smal", bufs=4))
            ps_band = st.enter_context(
                tc.tile_pool(name="ps_band", bufs=_bufs("ps_band", 2),
                             space="PSUM"))
            ps_sc = st.enter_context(
                tc.tile_pool(name="ps_sc", bufs=_bufs("ps_sc", 2),
                             space="PSUM"))
            ps_ctx = st.enter_context(
                tc.tile_pool(name="ps_ctx", bufs=1, space="PSUM"))
            ps_ctxT = st.enter_context(
                tc.tile_pool(name="ps_ctxT", bufs=1, space="PSUM"))

            for b in range(NB):
                tok0 = 512 * b
                outs = [outp.tile([128, D], f32, name=f"outs{i}",
                                  tag=f"outs{i}") for i in range(4)]
                for hp in range(H // 2):
                    # head pair (2hp, 2hp+1) = rows [0:64] / [64:128] of
                    # SBUF tile hp; K=64 matmuls at base partitions 0/64
                    # are issued adjacently so the PE overlaps them
                    # (row-strip concurrency).
                    th = hp
                    qh = QT[th]
                    kh = KT[th]
                    pkr = PKR[th]
                    pq = PQ[th]
                    RS = (slice(0, 64), slice(64, 128))

                    # --- band matmuls + DRAM roundtrip (both heads) ---
                    cb_d = [dram.tile([4, 128, BW], bdt, name=f"cb_d{s}",
                                      tag=f"cb{s}") for s in range(2)]
                    pb_d = [dram.tile([4, 128, BW], bdt, name=f"pb_d{s}",
                                      tag=f"pb{s}") for s in range(2)]
                    if "nobandmm" not in _ABL:
                        def _band_job(kind, idx, s, ps):
                            # emit the two MMs for one band tile; returns
                            # (evac_engine, dram_dest)
                            if kind == 0:
                                w0 = 384 - 128 * idx
                                lhsT = qh[RS[s], tok0 + 128 * idx :
                                          tok0 + 128 * (idx + 1)]
                                rhs = pkr
                            else:
                                w0 = 385 - 128 * idx
                                lhsT = kh[RS[s], tok0 + 128 * idx :
                                          tok0 + 128 * (idx + 1)]
                                rhs = pq
                            return lhsT, rhs, w0

                        def _band_mm(kind, idx, s, ps, half):
                            lhsT, rhs, w0 = _band_job(kind, idx, s, ps)
                            nc.tensor.matmul(
                                ps[:, half], lhsT,
                                rhs[RS[s], w0 + half.start : w0 + half.stop],
                                start=True, stop=True)

                        def _band_out(kind, idx, s, ps):
                            # split the PSUM evacuation across ACT and DVE
                            # so the PSUM slot frees ~2x faster
                            bb = bsb.tile([128, BW], bdt, name="bb",
                                          tag="bb")
                            nc.scalar.activation(bb[:, 0:352], ps[:, 0:352],
                                                 FT.Copy)
                            nc.vector.tensor_copy(bb[:, 352:BW],
                                                  ps[:, 352:BW])
                            dst = cb_d[s][idx] if kind == 0 else pb_d[s][idx]
                            if "nort" not in _ABL:
                                nc.sync.dma_start(dst, bb[:])

                        halves = (slice(0, 512), slice(512, BW))
                        if "nopair" in _ABL:
                            for s in range(2):
                                for kind in range(2):
                                    for idx in range(4):
                                        ps = ps_band.tile(
                                            [128, BW], f32, name="ps_b",
                                            tag="ps_band")
                                        for half in halves:
                                            _band_mm(kind, idx, s, ps, half)
                                        _band_out(kind, idx, s, ps)
                        else:
                            for kind in range(2):
                                for idx in range(4):
                                    pss = [ps_band.tile(
                                        [128, BW], f32, name=f"ps_b{s}",
                                        tag="ps_band") for s in range(2)]
                                    for half in halves:
                                        for s in range(2):
                                            _band_mm(kind, idx, s, pss[s],
                                                     half)
                                    for s in range(2):
                                        _band_out(kind, idx, s, pss[s])

                    # --- skewed (diagonal) re-reads (both heads) ---
                    cbr = [[], []]
                    pbr = [[], []]
                    skiprd = "nort" in _ABL or "nobandmm" in _ABL
                    for s in range(2):
                        for I in range(4):
                            t_ = brd.tile([128, 512], bdt, name=f"cbr{s}{I}",
                                          tag=f"cbr{s}{I}")
                            if not skiprd:
                                src = bass.AP(cb_d[s].tensor,
                                              cb_d[s].offset + I * 128 * BW + 127,
                                              [[BP, 128], [1, 512]])
                                nc.sync.dma_start(t_[:], src)
                            else:
                                nc.gpsimd.memset(t_[:], 0.0)
                            cbr[s].append(t_)
                        for J in range(4):
                            t_ = brd.tile([128, 512], bdt, name=f"pbr{s}{J}",
                                          tag=f"pbr{s}{J}")
                            if not skiprd:
                                src = bass.AP(pb_d[s].tensor,
                                              pb_d[s].offset + J * 128 * BW + 127,
                                              [[BP, 128], [1, 512]])
                                nc.sync.dma_start(t_[:], src)
                            else:
                                nc.gpsimd.memset(t_[:], 0.0)
                            pbr[s].append(t_)

                    # --- per head: scores^T, exp, PV, ctx ---
                    for s in range(2):
                        h = 2 * hp + s
                        rsl = RS[s]
                        exps = []
                        for J in range(4):
                            ps = ps_sc.tile([128, 512], f32, name="ps_s",
                                            tag="ps_s")
                            noadds = "noadds" in _ABL
                            nc.tensor.matmul(
                                ps[:],
                                kh[rsl, tok0 + 128 * J : tok0 + 128 * (J + 1)],
                                qh[rsl, tok0:tok0 + 512],
                                start=True, stop=noadds)
                            if not noadds:
                                nc.tensor.matmul(ps[:], idn[:], pbr[s][J][:],
                                                 start=False, stop=False)
                                for I in range(4):
                                    nc.tensor.matmul(
                                        ps[:, 128 * I : 128 * (I + 1)],
                                        cbr[s][I][:, 128 * J : 128 * (J + 1)],
                                        idn[:], start=False, stop=(I == 3))
                            e = expp.tile([128, 512], bf16, name=f"exps{J}",
                                          tag=f"exps{J}")
                            nc.scalar.activation(e[:], ps[:], FT.Exp)
                            exps.append(e)

                        if "nopv" in _ABL:
                            for Ic in range(4):
                                nc.vector.memset(
                                    outs[Ic][:, 64 * h : 64 * h + 64], 0.0)
                            continue
                        # --- PV (ones-augmented) ---
                        pc = ps_ctx.tile([65, 512], f32, name="pc", tag="pc")
                        for J in range(4):
                            nc.tensor.matmul(
                                pc[:], VA[4 * b + J][:, 65 * h : 65 * h + 65],
                                exps[J][:], start=(J == 0), stop=(J == 3))
                        ctxT = smal.tile([65, 512], f32, name="ctxT",
                                         tag="ctxT")
                        nc.vector.tensor_copy(ctxT[:], pc[:])

                        for Ic in range(4):
                            pt = ps_ctxT.tile([128, 65], f32, name="pt",
                                              tag="pt")
                            nc.tensor.transpose(
                                pt[:], ctxT[:, 128 * Ic : 128 * (Ic + 1)],
                                idnf[0:65, 0:65])
                            rec = smal.tile([128, 1], f32, name="rec",
                                            tag="rec")
                            nc.vector.reciprocal(rec[:], pt[:, 64:65])
                            nc.vector.tensor_scalar_mul(
                                outs[Ic][:, 64 * h : 64 * h + 64],
                                pt[:, 0:64], rec[:])

                for Ic in range(4):
                    nc.sync.dma_start(
                        out_d.ap()[tok0 + 128 * Ic : tok0 + 128 * (Ic + 1)],
                        outs[Ic][:])


_NC_CACHE = None
LAST = {}


def _get_nc():
    global _NC_CACHE
    if _NC_CACHE is None:
        _NC_CACHE = build_kernel()
    return _NC_CACHE


def kernel(q, k, v, rel_embeddings, Wq, bq, Wk, bk, Wv, bv, relative_pos,
           **_unused):
    q = np.asarray(q, np.float32)
    k = np.asarray(k, np.float32)
    v = np.asarray(v, np.float32)
    rel = np.asarray(rel_embeddings, np.float32)
    Wq = np.asarray(Wq, np.float32)
    Wk = np.asarray(Wk, np.float32)
    Wv = np.asarray(Wv, np.float32)
    bq = np.asarray(bq, np.float32)
    bk = np.asarray(bk, np.float32)
    bv = np.asarray(bv, np.float32)

    Wq_s, bq_s = Wq / SCALE, bq / SCALE
    wq_b = Wq_s.astype(_nbf)
    wk_b = Wk.astype(_nbf)
    wv_b = Wv.astype(_nbf)
    rT = np.ascontiguousarray(rel.T).astype(_nbf)
    rTr = np.ascontiguousarray(rel[::-1].T).astype(_nbf)
    biases = np.stack([bq_s.reshape(KC, 128), bk.reshape(KC, 128)], 0)
    biases = np.ascontiguousarray(
        biases.reshape(2 * KC, 128).T).astype(np.float32)  # [128, 2*KC]

    in_maps = []
    for c in range(NCORES):
        bs = [NB * c + i for i in range(NB)]
        qT = np.ascontiguousarray(
            np.concatenate([q[b].T for b in bs], axis=1)).astype(_nbf)
        kT = np.ascontiguousarray(
            np.concatenate([k[b].T for b in bs], axis=1)).astype(_nbf)
        vT = np.ascontiguousarray(
            np.concatenate([v[b].T for b in bs], axis=1)).astype(_nbf)
        in_maps.append({
            "qT": qT, "kT": kT, "vT": vT,
            "Wq": wq_b, "Wk": wk_b, "Wv": wv_b,
            "rT": rT, "rTrev": rTr, "biases": biases,
        })

    nc = _get_nc()
    res = bass_utils.run_bass_kernel_spmd(
        nc, in_maps, core_ids=list(range(NCORES)),
        trace=bool(int(os.environ.get("KTRACE", "0"))))
    LAST["res"] = res
    out = np.empty((B, S, D), np.float32)
    for c in range(NCORES):
        o = res.results[c]["out"].reshape(NB, S, D)
        for i in range(NB):
            out[NB * c + i] = o[i]
    return out


if __name__ == "__main__":
    nc = build_kernel()
    print("built ok")



# revision 5
# speedup vs baseline: 1.0780x; 1.0780x over previous
"""Trainium2 Bass kernel for DeBERTa-style disentangled attention.

Problem: B=16, S=512, D=768, H=12, HD=64, L=512 (att_span), scale=sqrt(3*64).

  Q = q@Wq+bq, K = k@Wk+bk, V = v@Wv+bv   (per-head split)
  scores = (QK^T + c2p + p2c) / scale ; softmax ; ctx = P@V
  c2p[i,j] = Q[i] . pos_k[i-j+512]   (pos_k = rel@Wk+bk, per head)
  p2c[i,j] = K[j] . pos_q[i-j+512]   (pos_q = rel@Wq+bq)
  (clip never binds: i-j+512 in [1,1023])

Sharding: data-parallel over batch, 8 cores x (B_local=2).

Device strategy (per core, everything transposed "scores^T[j,i]"):
  - Projections produce QT/KT [dout, tok] (bf16), V [tok, dout] (bf16,
    augmented with a ones-column per head for softmax denominators),
    PKR = pos_k_reversed^T and PQ = pos_q^T [dout, p] (bf16).
    1/scale folded into Wq/bq on host (covers c2c, c2p via Q; p2c via pos_q).
  - Per (b,h): band matmuls produce c2p_att_rev / p2c_att [128, 640] tiles,
    evacuated bf16 and round-tripped through DRAM; strided re-read
    (row-stride 639, offset 127) yields the diagonal-gathered
    c2p [i,j] / p2cT [j,i] tiles (the DeBERTa "skew" trick).
  - scores^T accumulated in PSUM: c2cT matmul + p2cT via identity-add +
    c2p via PE add-transpose (lhsT=c2p chunk, rhs=identity).
  - exp on ACT (no max subtraction needed: |scores| <~ 3), PV matmul with
    ones-augmented V gives ctx^T and row sums; PE transpose + DVE
    reciprocal/scale finishes ctx = P@V / sums in fp32.
"""

import os
import sys
import numpy as np

for p in ("/opt/trn_rl_repo",):
    if p not in sys.path:
        sys.path.insert(0, p)

import ml_dtypes

import concourse.bass as bass
import concourse.bacc as bacc
import concourse.tile as tile
import concourse.mybir as mybir
from concourse import bass_utils

f32 = mybir.dt.float32
bf16 = mybir.dt.bfloat16
fp8 = mybir.dt.float8e4
FT = mybir.ActivationFunctionType

B, S, D, H = 16, 512, 768, 12
HD = 64
L = 512
P2 = 2 * L  # 1024
NB = 2  # batches per core
NTOK = NB * S  # 1024
NCORES = 8
SCALE = float(np.sqrt(HD * 3.0))
KC = D // 128  # 6 contraction chunks
BW = 640  # band width (pads the 639 used diagonals)
BP = 639  # band row pitch for the skew read

_nbf = ml_dtypes.bfloat16

# ablation / tuning knobs (TimelineSim experiments)
_ABL = set(os.environ.get("KABL", "").split(",")) - {""}
_BUFS = {}
for _kv in os.environ.get("KBUFS", "").split(","):
    if _kv:
        _k, _v = _kv.split("=")
        _BUFS[_k] = int(_v)


def _bufs(name, default):
    return _BUFS.get(name, default)


def build_kernel(abl=None, bufs=None, nrep=1):
    global _ABL, _BUFS
    if abl is not None:
        _ABL = set(abl)
    if bufs is not None:
        _BUFS = dict(bufs)
    nc = bacc.Bacc(
        "TRN2",
        target_bir_lowering=False,
        debug=False,
        enable_asserts=False,
        num_devices=NCORES,
    )

    # ---- I/O ----
    qT_d = nc.dram_tensor("qT", [D, NTOK], bf16, kind="ExternalInput")
    kT_d = nc.dram_tensor("kT", [D, NTOK], bf16, kind="ExternalInput")
    vT_d = nc.dram_tensor("vT", [D, NTOK], bf16, kind="ExternalInput")
    wq_d = nc.dram_tensor("Wq", [D, D], bf16, kind="ExternalInput")  # pre-scaled
    wk_d = nc.dram_tensor("Wk", [D, D], bf16, kind="ExternalInput")
    wv_d = nc.dram_tensor("Wv", [D, D], bf16, kind="ExternalInput")
    rT_d = nc.dram_tensor("rT", [D, P2], bf16, kind="ExternalInput")
    rTr_d = nc.dram_tensor("rTrev", [D, P2], bf16, kind="ExternalInput")
    bias_d = nc.dram_tensor("biases", [128, 2 * KC], f32, kind="ExternalInput")
    out_d = nc.dram_tensor("out", [NTOK, D], f32, kind="ExternalOutput")

    idn_np = np.eye(128, dtype=_nbf)
    if "fp8rt" in _ABL:
        idn_np = np.eye(128, dtype=ml_dtypes.float8_e4m3)
    idn_d = nc.inline_tensor(idn_np, name="idn_bf")
    idnf_np = np.eye(128, dtype=np.float32)
    idnf_d = nc.inline_tensor(idnf_np, name="idn_f32")

    with tile.TileContext(nc) as tc:
        for _rep in range(nrep):
            _body(nc, tc, qT_d, kT_d, vT_d, wq_d, wk_d, wv_d, rT_d, rTr_d,
                  bias_d, idn_d, idnf_d, out_d)
    nc.compile()
    return nc


def _body(nc, tc, qT_d, kT_d, vT_d, wq_d, wk_d, wv_d, rT_d, rTr_d,
          bias_d, idn_d, idnf_d, out_d):
    from contextlib import ExitStack

    with ExitStack() as big:
        const = big.enter_context(tc.tile_pool(name="const", bufs=1))
        acts = big.enter_context(tc.tile_pool(name="acts", bufs=1))

        bdt = fp8 if "fp8rt" in _ABL else bf16
        idn = const.tile([128, 128], bdt)
        nc.sync.dma_start(idn[:], idn_d.ap())
        biases = const.tile([128, 2 * KC], f32)
        nc.sync.dma_start(biases[:], bias_d.ap())

        # persistent activations
        QT = [acts.tile([128, NTOK], bf16, name=f"QT{t}") for t in range(KC)]
        KT = [acts.tile([128, NTOK], bf16, name=f"KT{t}") for t in range(KC)]
        PKR = [acts.tile([128, P2 + 1], bf16, name=f"PKR{t}") for t in range(KC)]
        PQ = [acts.tile([128, P2 + 1], bf16, name=f"PQ{t}") for t in range(KC)]
        VA = [acts.tile([128, 65 * H], bf16, name=f"VA{c}") for c in range(8)]

        # ---------------- Stage P: projections ----------------
        with ExitStack() as st:
            inp = st.enter_context(tc.tile_pool(name="inp", bufs=1))
            psp = st.enter_context(
                tc.tile_pool(name="psp", bufs=4, space="PSUM"))

            qT = [inp.tile([128, NTOK], bf16, name=f"qT{t}") for t in range(KC)]
            kTt = [inp.tile([128, NTOK], bf16, name=f"kTt{t}") for t in range(KC)]
            vT = [inp.tile([128, NTOK], bf16, name=f"vT{t}") for t in range(KC)]
            rT = [inp.tile([128, P2], bf16, name=f"rT{t}") for t in range(KC)]
            rTr = [inp.tile([128, P2], bf16, name=f"rTr{t}") for t in range(KC)]
            for t in range(KC):
                sl = slice(128 * t, 128 * (t + 1))
                nc.sync.dma_start(qT[t][:], qT_d.ap()[sl])
                nc.sync.dma_start(kTt[t][:], kT_d.ap()[sl])
                nc.sync.dma_start(vT[t][:], vT_d.ap()[sl])
                nc.sync.dma_start(rT[t][:], rT_d.ap()[sl])
                nc.sync.dma_start(rTr[t][:], rTr_d.ap()[sl])

            wq = [inp.tile([128, D], bf16, name=f"wq{t}") for t in range(KC)]
            wk = [inp.tile([128, D], bf16, name=f"wk{t}") for t in range(KC)]
            wv = [inp.tile([128, D], bf16, name=f"wv{t}") for t in range(KC)]
            for t in range(KC):
                sl = slice(128 * t, 128 * (t + 1))
                nc.sync.dma_start(wq[t][:], wq_d.ap()[sl])
                nc.sync.dma_start(wk[t][:], wk_d.ap()[sl])
                nc.sync.dma_start(wv[t][:], wv_d.ap()[sl])

            # QT / KT / PKR / PQ : out[dout_tile, tok] = W^T @ xT (+ bias)
            if "noproj" in _ABL:
                for t in range(KC):
                    nc.vector.memset(QT[t][:], 0.0)
                    nc.vector.memset(KT[t][:], 0.0)
                    nc.vector.memset(PKR[t][:], 0.0)
                    nc.vector.memset(PQ[t][:], 0.0)
                for c in range(8):
                    nc.vector.memset(VA[c][:], 0.0)
            for t in range(KC if "noproj" not in _ABL else 0):
                wsl = slice(128 * t, 128 * (t + 1))
                for th in range(2):  # token/pos halves of 512
                    tsl = slice(512 * th, 512 * (th + 1))
                    for (wmat, xin, bcol, dst) in (
                        (wq, qT, 0, QT), (wk, kTt, 1, KT),
                        (wk, rTr, 1, PKR), (wq, rT, 0, PQ),
                    ):
                        ps = psp.tile([128, 512], f32, name="ps_proj",
                                      tag="ps_proj", bufs=4)
                        for kc in range(KC):
                            nc.tensor.matmul(
                                ps[:], wmat[kc][:, wsl], xin[kc][:, tsl],
                                start=(kc == 0), stop=(kc == KC - 1))
                        nc.scalar.activation(
                            dst[t][:, tsl], ps[:], FT.Identity,
                            bias=biases[:, bcol * KC + t : bcol * KC + t + 1],
                            scale=1.0)

            # garbage-pad column P2 of PKR/PQ: zero it
            for t in range(KC):
                nc.vector.memset(PKR[t][:, P2:P2 + 1], 0.0)
                nc.vector.memset(PQ[t][:, P2:P2 + 1], 0.0)

            # V (+ ones cols): out[tok_chunk, dout] = vT^T @ Wv
            for c in range(8 if "noproj" not in _ABL else 0):
                csl = slice(128 * c, 128 * (c + 1))
                ps = psp.tile([128, D], f32, name="ps_v", tag="ps_v", bufs=2)
                for osl in (slice(0, 512), slice(512, D)):
                    for kc in range(KC):
                        nc.tensor.matmul(
                            ps[:, osl], vT[kc][:, csl], wv[kc][:, osl],
                            start=(kc == 0), stop=(kc == KC - 1))
                # strided evac: VA[c][:, 65h + d] = ps[:, 64h + d]
                va_v = VA[c][:].rearrange("p (h c) -> p h c", c=65)
                ps_v = ps[:].rearrange("p (h c) -> p h c", c=64)
                nc.vector.tensor_copy(va_v[:, :, 0:64], ps_v)
                nc.vector.memset(va_v[:, :, 64:65], 1.0)

        # ---------------- Stage A: attention ----------------
        with ExitStack() as st:
            bsb = st.enter_context(
                tc.tile_pool(name="bsb", bufs=_bufs("bsb", 2)))
            brd = st.enter_context(
                tc.tile_pool(name="brd", bufs=_bufs("brd", 2)))
            expp = st.enter_context(
                tc.tile_pool(name="expp", bufs=_bufs("expp", 2)))
            outp = st.enter_context(tc.tile_pool(name="outp", bufs=2))
            smal = st.enter_context(tc.tile_pool(name="smal", bufs=4))
            ps_band = st.enter_context(
                tc.tile_pool(name="ps_band", bufs=_bufs("ps_band", 2),
                             space="PSUM"))
            ps_sc = st.enter_context(
                tc.tile_pool(name="ps_sc", bufs=_bufs("ps_sc", 3),
                             space="PSUM"))
            ps_ctx = st.enter_context(
                tc.tile_pool(name="ps_ctx", bufs=1, space="PSUM"))

            for b in range(NB):
                tok0 = 512 * b
                outs = [outp.tile([128, D], f32, name=f"outs{i}",
                                  tag=f"outs{i}") for i in range(4)]
                for hp in range(H // 2):
                    # head pair (2hp, 2hp+1) = rows [0:64] / [64:128] of
                    # SBUF tile hp; K=64 matmuls at base partitions 0/64
                    # are issued adjacently so the PE overlaps them
                    # (row-strip concurrency).
                    th = hp
                    qh = QT[th]
                    kh = KT[th]
                    pkr = PKR[th]
                    pq = PQ[th]
                    RS = (slice(0, 64), slice(64, 128))

                    # --- band matmuls (both heads), fused per (kind,s) ---
                    # bb[kind][s] holds 4 idx-tiles side by side:
                    # bb[p, 640*idx + c] = band value (query/key p of chunk
                    # idx, window position c)
                    bb = [[bsb.tile([128, 4 * BW], bdt, name=f"bb{k}{s}",
                                    tag=f"bb{k}{s}") for s in range(2)]
                          for k in range(2)]

                    def _band_mm(kind, idx, s, half):
                        if kind == 0:
                            w0 = 384 - 128 * idx
                            lhsT = qh[RS[s], tok0 + 128 * idx :
                                      tok0 + 128 * (idx + 1)]
                            rhs = pkr
                        else:
                            w0 = 385 - 128 * idx
                            lhsT = kh[RS[s], tok0 + 128 * idx :
                                      tok0 + 128 * (idx + 1)]
                            rhs = pq
                        nc.tensor.matmul(
                            pss[s][:, half], lhsT,
                            rhs[RS[s], w0 + half.start : w0 + half.stop],
                            start=True, stop=True)

                    halves = (slice(0, 512), slice(512, BW))
                    for kind in range(2):
                        for idx in range(4):
                            pss = [ps_band.tile(
                                [128, BW], f32, name=f"ps_b{s}",
                                tag="ps_band") for s in range(2)]
                            for half in halves:
                                for s in range(2):
                                    _band_mm(kind, idx, s, half)
                            for s in range(2):
                                # split PSUM evac across ACT and DVE so the
                                # slot frees ~2x faster
                                dst = bb[kind][s]
                                c0 = BW * idx
                                nc.scalar.activation(
                                    dst[:, c0:c0 + 352], pss[s][:, 0:352],
                                    FT.Copy)
                                nc.vector.tensor_copy(
                                    dst[:, c0 + 352:c0 + BW],
                                    pss[s][:, 352:BW])

                    # --- skewed (diagonal) re-reads: one SBUF->SBUF DMA
                    # per (kind, s).  out[p, 512*idx + j] = bb[p,
                    # 640*idx + 127 - p + j]  (the DeBERTa shear).
                    cbr = []
                    pbr = []
                    for s in range(2):
                        t_ = brd.tile([128, 2048], bdt, name=f"cbr{s}",
                                      tag=f"cbr{s}")
                        src = bass.AP(bb[0][s].tensor, bb[0][s].offset + 127,
                                      [[4 * BW - 1, 128], [BW, 4], [1, 512]])
                        nc.sync.dma_start(t_[:], src)
                        cbr.append(t_)
                        t_ = brd.tile([128, 2048], bdt, name=f"pbr{s}",
                                      tag=f"pbr{s}")
                        src = bass.AP(bb[1][s].tensor, bb[1][s].offset + 127,
                                      [[4 * BW - 1, 128], [BW, 4], [1, 512]])
                        nc.sync.dma_start(t_[:], src)
                        pbr.append(t_)

                    # --- scores^T tiles: c2c (paired strips) + adds + exp
                    exps = [[None] * 4 for _ in range(2)]
                    for J in range(4):
                        pss = [ps_sc.tile([128, 512], f32, name=f"ps_s{s}",
                                          tag="ps_s") for s in range(2)]
                        for s in range(2):
                            nc.tensor.matmul(
                                pss[s][:],
                                kh[RS[s], tok0 + 128 * J : tok0 + 128 * (J + 1)],
                                qh[RS[s], tok0:tok0 + 512],
                                start=True, stop=False)
                        for s in range(2):
                            ps = pss[s]
                            nc.tensor.matmul(
                                ps[:], idn[:],
                                pbr[s][:, 512 * J : 512 * (J + 1)],
                                start=False, stop=False)
                            for I in range(4):
                                nc.tensor.matmul(
                                    ps[:, 128 * I : 128 * (I + 1)],
                                    cbr[s][:, 512 * I + 128 * J :
                                            512 * I + 128 * (J + 1)],
                                    idn[:], start=False, stop=(I == 3))
                            e = expp.tile([128, 512], bf16, name=f"exps{s}{J}",
                                          tag=f"exps{s}{J}")
                            nc.scalar.activation(e[:], ps[:], FT.Exp)
                            exps[s][J] = e

                    # --- PV direct: ctx[i, d] per 128-query chunk ---
                    for s in range(2):
                        h = 2 * hp + s
                        for Ic in range(4):
                            po = ps_ctx.tile([128, 65], f32, name="po",
                                             tag="po")
                            for J in range(4):
                                nc.tensor.matmul(
                                    po[:],
                                    exps[s][J][:, 128 * Ic : 128 * (Ic + 1)],
                                    VA[4 * b + J][:, 65 * h : 65 * h + 65],
                                    start=(J == 0), stop=(J == 3))
                            rec = smal.tile([128, 1], f32, name="rec",
                                            tag="rec")
                            nc.vector.reciprocal(rec[:], po[:, 64:65])
                            nc.vector.tensor_scalar_mul(
                                outs[Ic][:, 64 * h : 64 * h + 64],
                                po[:, 0:64], rec[:])

                for Ic in range(4):
                    nc.sync.dma_start(
                        out_d.ap()[tok0 + 128 * Ic : tok0 + 128 * (Ic + 1)],
                        outs[Ic][:])


_NC_CACHE = None
LAST = {}


def _get_nc():
    global _NC_CACHE
    if _NC_CACHE is None:
        _NC_CACHE = build_kernel()
    return _NC_CACHE


def kernel(q, k, v, rel_embeddings, Wq, bq, Wk, bk, Wv, bv, relative_pos,
           **_unused):
    q = np.asarray(q, np.float32)
    k = np.asarray(k, np.float32)
    v = np.asarray(v, np.float32)
    rel = np.asarray(rel_embeddings, np.float32)
    Wq = np.asarray(Wq, np.float32)
    Wk = np.asarray(Wk, np.float32)
    Wv = np.asarray(Wv, np.float32)
    bq = np.asarray(bq, np.float32)
    bk = np.asarray(bk, np.float32)
    bv = np.asarray(bv, np.float32)

    Wq_s, bq_s = Wq / SCALE, bq / SCALE
    wq_b = Wq_s.astype(_nbf)
    wk_b = Wk.astype(_nbf)
    wv_b = Wv.astype(_nbf)
    rT = np.ascontiguousarray(rel.T).astype(_nbf)
    rTr = np.ascontiguousarray(rel[::-1].T).astype(_nbf)
    biases = np.stack([bq_s.reshape(KC, 128), bk.reshape(KC, 128)], 0)
    biases = np.ascontiguousarray(
        biases.reshape(2 * KC, 128).T).astype(np.float32)  # [128, 2*KC]

    in_maps = []
    for c in range(NCORES):
        bs = [NB * c + i for i in range(NB)]
        qT = np.ascontiguousarray(
            np.concatenate([q[b].T for b in bs], axis=1)).astype(_nbf)
        kT = np.ascontiguousarray(
            np.concatenate([k[b].T for b in bs], axis=1)).astype(_nbf)
        vT = np.ascontiguousarray(
            np.concatenate([v[b].T for b in bs], axis=1)).astype(_nbf)
        in_maps.append({
            "qT": qT, "kT": kT, "vT": vT,
            "Wq": wq_b, "Wk": wk_b, "Wv": wv_b,
            "rT": rT, "rTrev": rTr, "biases": biases,
        })

    nc = _get_nc()
    res = bass_utils.run_bass_kernel_spmd(
        nc, in_maps, core_ids=list(range(NCORES)),
        trace=bool(int(os.environ.get("KTRACE", "0"))))
    LAST["res"] = res
    out = np.empty((B, S, D), np.float32)
    for c in range(NCORES):
        o = res.results[c]["out"].reshape(NB, S, D)
        for i in range(NB):
            out[NB * c + i] = o[i]
    return out


if __name__ == "__main__":
    nc = build_kernel()
    print("built ok")



# revision 7
# speedup vs baseline: 1.3303x; 1.2340x over previous
"""Trainium2 Bass kernel for DeBERTa-style disentangled attention.

Problem: B=16, S=512, D=768, H=12, HD=64, L=512 (att_span), scale=sqrt(3*64).

  Q = q@Wq+bq, K = k@Wk+bk, V = v@Wv+bv   (per-head split)
  scores = (QK^T + c2p + p2c) / scale ; softmax ; ctx = P@V
  c2p[i,j] = Q[i] . pos_k[i-j+512]   (pos_k = rel@Wk+bk, per head)
  p2c[i,j] = K[j] . pos_q[i-j+512]   (pos_q = rel@Wq+bq)
  (clip never binds: i-j+512 in [1,1023])

Sharding: data-parallel over batch, 8 cores x (B_local=2).

Device strategy (per core, everything transposed "scores^T[j,i]"):
  - Projections produce zero-padded per-head-pair activations: qz/kz
    [dout, tok] where the OTHER head's 64 rows are zero, so every
    matmul contracts over K=128 (K=64 matmuls stream ~1.5x slower).
    V [tok, dout] (bf16, + ones-column per head for softmax sums).
    PKR = pos_k_reversed^T and PQ = pos_q^T [dout, p] stay full.
    1/scale folded into Wq/bq on host.
  - Per head-pair: band matmuls produce c2p_att_rev / p2c_att
    [128, 4*640] fp8 tiles; a single SBUF->SBUF strided DMA per
    (kind, head) performs the DeBERTa diagonal-shear gather
    (row p reads at offset 127 - p), yielding c2p [i,j] / p2cT [j,i].
  - scores^T accumulated in PSUM: c2cT matmul (K=128, zero-padded kz
    lhsT) + c2p via PE add-transpose (lhsT=c2p chunk fp8, rhs=idn fp8);
    p2cT added by DVE (scalar_tensor_tensor) directly in PSUM.
  - Band matmuls for head-pair X+1 are issued before scores of pair X
    (software pipelining hides the evac+shear latency).
  - exp on ACT (no max subtraction: |scores| <~ 3), PV with
    ones-augmented V gives ctx[i,d] directly (lhsT=exp chunk):
    DVE reciprocal/scale finishes ctx = P@V / sums in fp32.
"""

import os
import sys
import numpy as np

for p in ("/opt/trn_rl_repo",):
    if p not in sys.path:
        sys.path.insert(0, p)

import ml_dtypes

import concourse.bass as bass
import concourse.bacc as bacc
import concourse.tile as tile
import concourse.mybir as mybir
from concourse import bass_utils

f32 = mybir.dt.float32
bf16 = mybir.dt.bfloat16
fp8 = mybir.dt.float8e4
FT = mybir.ActivationFunctionType

B, S, D, H = 16, 512, 768, 12
HD = 64
L = 512
P2 = 2 * L  # 1024
NB = 2  # batches per core
NTOK = NB * S  # 1024
NCORES = 8
SCALE = float(np.sqrt(HD * 3.0))
KC = D // 128  # 6 contraction chunks
BW = 640  # band width (pads the 639 used diagonals)

_nbf = ml_dtypes.bfloat16

# ablation / tuning knobs (TimelineSim experiments)
_ABL = set(os.environ.get("KABL", "").split(",")) - {""}
_BUFS = {}
for _kv in os.environ.get("KBUFS", "").split(","):
    if _kv:
        _k, _v = _kv.split("=")
        _BUFS[_k] = int(_v)


def _bufs(name, default):
    return _BUFS.get(name, default)


def build_kernel(abl=None, bufs=None, nrep=1):
    global _ABL, _BUFS
    if abl is not None:
        _ABL = set(abl)
    if bufs is not None:
        _BUFS = dict(bufs)
    nc = bacc.Bacc(
        "TRN2",
        target_bir_lowering=False,
        debug=False,
        enable_asserts=False,
        num_devices=NCORES,
    )

    # ---- I/O ----
    qT_d = nc.dram_tensor("qT", [D, NTOK], bf16, kind="ExternalInput")
    kT_d = nc.dram_tensor("kT", [D, NTOK], bf16, kind="ExternalInput")
    vT_d = nc.dram_tensor("vT", [D, NTOK], bf16, kind="ExternalInput")
    wq_d = nc.dram_tensor("Wq", [D, D], bf16, kind="ExternalInput")  # pre-scaled
    wk_d = nc.dram_tensor("Wk", [D, D], bf16, kind="ExternalInput")
    wv_d = nc.dram_tensor("Wv", [D, D], bf16, kind="ExternalInput")
    rT_d = nc.dram_tensor("rT", [D, P2], bf16, kind="ExternalInput")
    rTr_d = nc.dram_tensor("rTrev", [D, P2], bf16, kind="ExternalInput")
    bias_d = nc.dram_tensor("biases", [128, 2 * KC], f32, kind="ExternalInput")
    out_d = nc.dram_tensor("out", [NTOK, D], f32, kind="ExternalOutput")

    idn_d = nc.inline_tensor(np.eye(128, dtype=ml_dtypes.float8_e4m3),
                             name="idn_f8")

    with tile.TileContext(nc) as tc:
        for _rep in range(nrep):
            _body(nc, tc, qT_d, kT_d, vT_d, wq_d, wk_d, wv_d, rT_d, rTr_d,
                  bias_d, idn_d, out_d)
    nc.compile()
    return nc


def _body(nc, tc, qT_d, kT_d, vT_d, wq_d, wk_d, wv_d, rT_d, rTr_d,
          bias_d, idn_d, out_d):
    from contextlib import ExitStack

    with ExitStack() as big:
        const = big.enter_context(tc.tile_pool(name="const", bufs=1))
        acts = big.enter_context(tc.tile_pool(name="acts", bufs=1))

        idn = const.tile([128, 128], fp8)
        nc.sync.dma_start(idn[:], idn_d.ap())
        biases = const.tile([128, 2 * KC], f32)
        nc.sync.dma_start(biases[:], bias_d.ap())

        # persistent activations; qz/kz[t][s]: head s of pair t in rows
        # [64s, 64s+64), other 64 rows zero -> K=128 matmuls stay exact
        QZ = [[acts.tile([128, NTOK], bf16, name=f"QZ{t}{s}")
               for s in range(2)] for t in range(KC)]
        KZ = [[acts.tile([128, NTOK], bf16, name=f"KZ{t}{s}")
               for s in range(2)] for t in range(KC)]
        PKR = [acts.tile([128, P2 + 1], bf16, name=f"PKR{t}") for t in range(KC)]
        PQ = [acts.tile([128, P2 + 1], bf16, name=f"PQ{t}") for t in range(KC)]
        VA = [acts.tile([128, 65 * H], bf16, name=f"VA{c}") for c in range(8)]

        RS = (slice(0, 64), slice(64, 128))
        for t in range(KC):
            for s in range(2):
                nc.gpsimd.memset(QZ[t][s][RS[1 - s], :], 0.0)
                nc.gpsimd.memset(KZ[t][s][RS[1 - s], :], 0.0)

        # ---------------- Stage P: projections ----------------
        with ExitStack() as st:
            inp = st.enter_context(tc.tile_pool(name="inp", bufs=1))
            psp = st.enter_context(
                tc.tile_pool(name="psp", bufs=4, space="PSUM"))

            qT = [inp.tile([128, NTOK], bf16, name=f"qT{t}") for t in range(KC)]
            kTt = [inp.tile([128, NTOK], bf16, name=f"kTt{t}") for t in range(KC)]
            vT = [inp.tile([128, NTOK], bf16, name=f"vT{t}") for t in range(KC)]
            rT = [inp.tile([128, P2], bf16, name=f"rT{t}") for t in range(KC)]
            rTr = [inp.tile([128, P2], bf16, name=f"rTr{t}") for t in range(KC)]
            for t in range(KC):
                sl = slice(128 * t, 128 * (t + 1))
                nc.sync.dma_start(qT[t][:], qT_d.ap()[sl])
                nc.sync.dma_start(kTt[t][:], kT_d.ap()[sl])
                nc.sync.dma_start(vT[t][:], vT_d.ap()[sl])
                nc.sync.dma_start(rT[t][:], rT_d.ap()[sl])
                nc.sync.dma_start(rTr[t][:], rTr_d.ap()[sl])

            wq = [inp.tile([128, D], bf16, name=f"wq{t}") for t in range(KC)]
            wk = [inp.tile([128, D], bf16, name=f"wk{t}") for t in range(KC)]
            wv = [inp.tile([128, D], bf16, name=f"wv{t}") for t in range(KC)]
            for t in range(KC):
                sl = slice(128 * t, 128 * (t + 1))
                nc.sync.dma_start(wq[t][:], wq_d.ap()[sl])
                nc.sync.dma_start(wk[t][:], wk_d.ap()[sl])
                nc.sync.dma_start(wv[t][:], wv_d.ap()[sl])

            # QZ / KZ / PKR / PQ : out[dout_tile, tok] = W^T @ xT (+ bias)
            for t in range(KC):
                wsl = slice(128 * t, 128 * (t + 1))
                for th in range(2):  # token/pos halves of 512
                    tsl = slice(512 * th, 512 * (th + 1))
                    for (wmat, xin, bcol, dst, split) in (
                        (wq, qT, 0, QZ[t], True), (wk, kTt, 1, KZ[t], True),
                        (wk, rTr, 1, PKR[t], False), (wq, rT, 0, PQ[t], False),
                    ):
                        ps = psp.tile([128, 512], f32, name="ps_proj",
                                      tag="ps_proj", bufs=4)
                        for kc in range(KC):
                            nc.tensor.matmul(
                                ps[:], wmat[kc][:, wsl], xin[kc][:, tsl],
                                start=(kc == 0), stop=(kc == KC - 1))
                        bc = bcol * KC + t
                        if split:
                            for s in range(2):
                                nc.scalar.activation(
                                    dst[s][RS[s], tsl], ps[RS[s], :],
                                    FT.Identity,
                                    bias=biases[RS[s], bc:bc + 1], scale=1.0)
                        else:
                            nc.scalar.activation(
                                dst[:, tsl], ps[:], FT.Identity,
                                bias=biases[:, bc:bc + 1], scale=1.0)

            # garbage-pad column P2 of PKR/PQ: zero it
            for t in range(KC):
                nc.vector.memset(PKR[t][:, P2:P2 + 1], 0.0)
                nc.vector.memset(PQ[t][:, P2:P2 + 1], 0.0)

            # V (+ ones cols): out[tok_chunk, dout] = vT^T @ Wv
            for c in range(8):
                csl = slice(128 * c, 128 * (c + 1))
                ps = psp.tile([128, D], f32, name="ps_v", tag="ps_v", bufs=2)
                for osl in (slice(0, 512), slice(512, D)):
                    for kc in range(KC):
                        nc.tensor.matmul(
                            ps[:, osl], vT[kc][:, csl], wv[kc][:, osl],
                            start=(kc == 0), stop=(kc == KC - 1))
                # strided evac: VA[c][:, 65h + d] = ps[:, 64h + d]
                va_v = VA[c][:].rearrange("p (h c) -> p h c", c=65)
                ps_v = ps[:].rearrange("p (h c) -> p h c", c=64)
                nc.vector.tensor_copy(va_v[:, :, 0:64], ps_v)
                nc.vector.memset(va_v[:, :, 64:65], 1.0)

        # ---------------- Stage A: attention ----------------
        with ExitStack() as st:
            bsb = st.enter_context(
                tc.tile_pool(name="bsb", bufs=_bufs("bsb", 2)))
            brd = st.enter_context(
                tc.tile_pool(name="brd", bufs=_bufs("brd", 2)))
            expp = st.enter_context(
                tc.tile_pool(name="expp", bufs=_bufs("expp", 2)))
            outp = st.enter_context(tc.tile_pool(name="outp", bufs=2))
            smal = st.enter_context(tc.tile_pool(name="smal", bufs=4))
            ps_band = st.enter_context(
                tc.tile_pool(name="ps_band", bufs=_bufs("ps_band", 2),
                             space="PSUM"))
            ps_sc = st.enter_context(
                tc.tile_pool(name="ps_sc", bufs=_bufs("ps_sc", 3),
                             space="PSUM"))
            ps_ctx = st.enter_context(
                tc.tile_pool(name="ps_ctx", bufs=1, space="PSUM"))

            def emit_band(b, hp):
                """Band matmuls + evac + shear DMAs for head pair hp of
                batch b.  Returns (cbr, pbr): per-head sheared tiles
                [128, 4*512] fp8: cbr[s][p, 512*I + j] = c2p[128I+p, j],
                pbr[s][p, 512*J + i] = p2cT[128J+p, i]."""
                tok0 = 512 * b
                th = hp
                bb = [[bsb.tile([128, 4 * BW], fp8, name=f"bb{k}{s}",
                                tag=f"bb{k}{s}") for s in range(2)]
                      for k in range(2)]
                for kind in range(2):
                    for idx in range(4):
                        pss = [ps_band.tile(
                            [128, BW], f32, name=f"ps_b{s}",
                            tag="ps_band") for s in range(2)]
                        for half in (slice(0, 512), slice(512, BW)):
                            for s in range(2):
                                if kind == 0:
                                    w0 = 384 - 128 * idx
                                    lhsT = QZ[th][s][:, tok0 + 128 * idx :
                                                     tok0 + 128 * (idx + 1)]
                                    rhs = PKR[th]
                                else:
                                    w0 = 385 - 128 * idx
                                    lhsT = KZ[th][s][:, tok0 + 128 * idx :
                                                     tok0 + 128 * (idx + 1)]
                                    rhs = PQ[th]
                                nc.tensor.matmul(
                                    pss[s][:, half], lhsT,
                                    rhs[:, w0 + half.start : w0 + half.stop],
                                    start=True, stop=True)
                        for s in range(2):
                            # split PSUM evac across ACT and DVE so the
                            # slot frees ~2x faster
                            dst = bb[kind][s]
                            c0 = BW * idx
                            nc.scalar.activation(
                                dst[:, c0:c0 + 352], pss[s][:, 0:352],
                                FT.Copy)
                            nc.vector.tensor_copy(
                                dst[:, c0 + 352:c0 + BW],
                                pss[s][:, 352:BW])

                # skewed (diagonal) re-read: one SBUF->SBUF DMA per
                # (kind, s): out[p, 512*idx + j] = bb[p, 640*idx+127-p+j]
                cbr, pbr = [], []
                for s in range(2):
                    for kind, lst in ((0, cbr), (1, pbr)):
                        t_ = brd.tile([128, 2048], fp8,
                                      name=f"shr{kind}{s}",
                                      tag=f"shr{kind}{s}")
                        src = bass.AP(
                            bb[kind][s].tensor, bb[kind][s].offset + 127,
                            [[4 * BW - 1, 128], [BW, 4], [1, 512]])
                        nc.sync.dma_start(t_[:], src)
                        lst.append(t_)
                return cbr, pbr

            def emit_scores(b, hp, cbr, pbr, outs):
                """scores^T, exp, PV, ctx for head pair hp of batch b."""
                tok0 = 512 * b
                th = hp
                exps = [[None] * 4 for _ in range(2)]
                for J in range(4):
                    pss = [ps_sc.tile([128, 512], f32, name=f"ps_s{s}",
                                      tag="ps_s") for s in range(2)]
                    for s in range(2):
                        nc.tensor.matmul(
                            pss[s][:],
                            KZ[th][s][:, tok0 + 128 * J : tok0 + 128 * (J + 1)],
                            QZ[th][s][:, tok0:tok0 + 512],
                            start=True, stop=False)
                    for s in range(2):
                        ps = pss[s]
                        for I in range(4):
                            nc.tensor.matmul(
                                ps[:, 128 * I : 128 * (I + 1)],
                                cbr[s][:, 512 * I + 128 * J :
                                        512 * I + 128 * (J + 1)],
                                idn[:], start=False, stop=(I == 3))
                        # p2cT add on DVE (PSUM in/out)
                        nc.vector.scalar_tensor_tensor(
                            ps[:], ps[:], 1.0,
                            pbr[s][:, 512 * J : 512 * (J + 1)],
                            op0=mybir.AluOpType.mult,
                            op1=mybir.AluOpType.add)
                        e = expp.tile([128, 512], bf16, name=f"exps{s}{J}",
                                      tag=f"exps{s}{J}")
                        nc.scalar.activation(e[:], ps[:], FT.Exp)
                        exps[s][J] = e

                # PV direct: ctx[i, d] per 128-query chunk
                for s in range(2):
                    h = 2 * hp + s
                    for Ic in range(4):
                        po = ps_ctx.tile([128, 65], f32, name="po",
                                         tag="po")
                        for J in range(4):
                            nc.tensor.matmul(
                                po[:],
                                exps[s][J][:, 128 * Ic : 128 * (Ic + 1)],
                                VA[4 * b + J][:, 65 * h : 65 * h + 65],
                                start=(J == 0), stop=(J == 3))
                        rec = smal.tile([128, 1], f32, name="rec",
                                        tag="rec")
                        nc.vector.reciprocal(rec[:], po[:, 64:65])
                        nc.vector.tensor_scalar_mul(
                            outs[Ic][:, 64 * h : 64 * h + 64],
                            po[:, 0:64], rec[:])

            # software pipeline: bands for pair X+1 issue before scores X
            allout = []
            for b in range(NB):
                allout.append([outp.tile([128, D], f32, name=f"outs{b}{i}",
                                         tag=f"outs{b}{i}") for i in range(4)])
            pairs = [(b, hp) for b in range(NB) for hp in range(H // 2)]
            pend = None
            for (b, hp) in pairs:
                cur = emit_band(b, hp)
                if pend is not None:
                    pb, php, pcbr, ppbr = pend
                    emit_scores(pb, php, pcbr, ppbr, allout[pb])
                    if php == H // 2 - 1:
                        for Ic in range(4):
                            nc.sync.dma_start(
                                out_d.ap()[512 * pb + 128 * Ic :
                                           512 * pb + 128 * (Ic + 1)],
                                allout[pb][Ic][:])
                pend = (b, hp, cur[0], cur[1])
            b, hp, cbr, pbr = pend
            emit_scores(b, hp, cbr, pbr, allout[b])
            for Ic in range(4):
                nc.sync.dma_start(
                    out_d.ap()[512 * b + 128 * Ic : 512 * b + 128 * (Ic + 1)],
                    allout[b][Ic][:])


_NC_CACHE = None
LAST = {}


def _get_nc():
    global _NC_CACHE
    if _NC_CACHE is None:
        _NC_CACHE = build_kernel()
    return _NC_CACHE


def kernel(q, k, v, rel_embeddings, Wq, bq, Wk, bk, Wv, bv, relative_pos,
           **_unused):
    q = np.asarray(q, np.float32)
    k = np.asarray(k, np.float32)
    v = np.asarray(v, np.float32)
    rel = np.asarray(rel_embeddings, np.float32)
    Wq = np.asarray(Wq, np.float32)
    Wk = np.asarray(Wk, np.float32)
    Wv = np.asarray(Wv, np.float32)
    bq = np.asarray(bq, np.float32)
    bk = np.asarray(bk, np.float32)
    bv = np.asarray(bv, np.float32)

    Wq_s, bq_s = Wq / SCALE, bq / SCALE
    wq_b = Wq_s.astype(_nbf)
    wk_b = Wk.astype(_nbf)
    wv_b = Wv.astype(_nbf)
    rT = np.ascontiguousarray(rel.T).astype(_nbf)
    rTr = np.ascontiguousarray(rel[::-1].T).astype(_nbf)
    biases = np.stack([bq_s.reshape(KC, 128), bk.reshape(KC, 128)], 0)
    biases = np.ascontiguousarray(
        biases.reshape(2 * KC, 128).T).astype(np.float32)  # [128, 2*KC]

    in_maps = []
    for c in range(NCORES):
        bs = [NB * c + i for i in range(NB)]
        qT = np.ascontiguousarray(
            np.concatenate([q[b].T for b in bs], axis=1)).astype(_nbf)
        kT = np.ascontiguousarray(
            np.concatenate([k[b].T for b in bs], axis=1)).astype(_nbf)
        vT = np.ascontiguousarray(
            np.concatenate([v[b].T for b in bs], axis=1)).astype(_nbf)
        in_maps.append({
            "qT": qT, "kT": kT, "vT": vT,
            "Wq": wq_b, "Wk": wk_b, "Wv": wv_b,
            "rT": rT, "rTrev": rTr, "biases": biases,
        })

    nc = _get_nc()
    res = bass_utils.run_bass_kernel_spmd(
        nc, in_maps, core_ids=list(range(NCORES)),
        trace=bool(int(os.environ.get("KTRACE", "0"))))
    LAST["res"] = res
    out = np.empty((B, S, D), np.float32)
    for c in range(NCORES):
        o = res.results[c]["out"].reshape(NB, S, D)
        for i in range(NB):
            out[NB * c + i] = o[i]
    return out


if __name__ == "__main__":
    nc = build_kernel()
    print("built ok")
